# revision 4
# baseline (speedup 1.0000x reference)
"""Trainium2 Bass kernel for nn_Model1 (lag-weighted long-run covariance + MLP).

Math: the 129-lag weighted covariance collapses algebraically:
    sum_l w_l * (Xc @ Y_l.T) = Xc @ (sum_l w_l Y_l).T
where Y_l is the (masked) l-shift of Xc.  So cov = (Xc @ P.T + N @ Xc.T)/d
with P, N two 65-tap causal FIR filters of Xc.  Centering is pushed through
the filters as rank-1 corrections so all GEMMs run on UNCENTERED X:
    cov.T * d = U.T - m (x) alpha - beta (x) m
with U = X@P0.T + N0@X.T (P0,N0 = filters of raw X), m = row means, and
alpha/beta derived from 5 reduction vectors (r,a,c from aux weights; p,q =
column sums of the filtered blocks, via ones-stationary matmuls).

Distribution (8 cores):
  - cov stage: shard time axis (512 cols/core), one AllReduce of [261,256]
    bf16; X^T is loaded once per core in a core-rotated block layout
    (slots 0-3 local, slot 4 halo, 5-32 the rest) so stage 1 starts early.
  - MLP: tensor-parallel over hidden (512/core), AllGather (bf16) between
    fc1->fc2 and fc2->fc3; fc3 emits batch-major so proj shards the output
    columns; final gather is a host-side concat.
Scheduling: w2/w3 weight streams are gated (WAW dep) to start only after the
AllReduce / first AllGather so collective mesh DMAs don't starve behind
weight traffic; all gather/scatter bounce copies are single large DMAs.
All heavy GEMMs use bf16 operands with fp32 PSUM accumulation.
"""
import math
import numpy as np
import ml_dtypes

NCORES = 8
Q = 64
NN = 256          # n (batch/rows of X)
DD = 4096         # d (time axis)
HID = 4096
Y0 = 512
HSH = HID // NCORES    # 512 hidden shard per core
NBLK = DD // 128       # 32 time blocks
BPC = NBLK // NCORES   # 4 blocks per core
NSLOT = NBLK + 1       # 33 xt slots (4 local, 1 halo, 28 rest)
KB = HID // 128        # 32 fc2/fc3 contraction blocks

bf16 = ml_dtypes.bfloat16

_CACHE = {}


# ----------------------------------------------------------------------------
# host-side weight-only precompute
# ----------------------------------------------------------------------------
def _erf(x):
    return np.vectorize(math.erf, otypes=[np.float64])(x)


def _gelu64(x):
    return 0.5 * x * (1.0 + _erf(x / np.sqrt(2.0)))


def _filters(inputs):
    f64 = lambda k: np.asarray(inputs[k], np.float64)
    lags = np.arange(-Q, Q + 1, dtype=np.float64)[:, None]
    h = _gelu64(lags @ f64("wn_w1") + f64("wn_b1"))
    w = (h @ f64("wn_w2") + f64("wn_b2"))[:, 0]
    wp = w[Q:]                                    # l = 0..Q
    wnv = np.concatenate([[0.0], w[:Q][::-1]])    # wnv[l] = w[Q-l], l=1..Q
    v = np.arange(128)[:, None]
    u = np.arange(128)[None, :]
    dvu = v - u
    d2 = dvu + 128
    A0 = np.where((dvu >= 0) & (dvu <= Q), wp[np.clip(dvu, 0, Q)], 0.0)
    A1 = np.where((d2 >= 0) & (d2 <= Q), wp[np.clip(d2, 0, Q)], 0.0)
    B0 = np.where((dvu >= 1) & (dvu <= Q), wnv[np.clip(dvu, 0, Q)], 0.0)
    B1 = np.where((d2 >= 1) & (d2 <= Q), wnv[np.clip(d2, 0, Q)], 0.0)
    t = np.arange(DD)
    lim = np.minimum(Q, DD - 1 - t)
    g_p = np.cumsum(wp)[lim]
    g_n = np.cumsum(wnv)[lim]
    gamma = float(g_p.sum() + g_n.sum())
    return A0, A1, B0, B1, g_p, g_n, gamma


# ----------------------------------------------------------------------------
# bass program
# ----------------------------------------------------------------------------
def build(gamma: float, use_gelu=True):
    import concourse.bacc as bacc
    import concourse.tile as tile
    import concourse.mybir as mybir

    dt32 = mybir.dt.float32
    dt16 = mybir.dt.bfloat16
    GELU = (mybir.ActivationFunctionType.Gelu if use_gelu
            else mybir.ActivationFunctionType.Identity)

    nc = bacc.Bacc("TRN2", target_bir_lowering=False, debug=False,
                   num_devices=NCORES)
    mm = nc.tensor.matmul
    RG = [list(range(NCORES))]

    # ---- I/O ----
    xt_d = nc.dram_tensor("xt", [128, NSLOT * 256], dt16, kind="ExternalInput").ap()
    tp_d = nc.dram_tensor("tp", [128, 512], dt16, kind="ExternalInput").ap()
    aux_d = nc.dram_tensor("aux", [128, BPC * 3], dt16, kind="ExternalInput").ap()
    w1_d = nc.dram_tensor("w1", [128, 64 * 512], dt16, kind="ExternalInput").ap()
    w2_d = nc.dram_tensor("w2", [128, KB * 512], dt16, kind="ExternalInput").ap()
    w3_d = nc.dram_tensor("w3", [128, KB * 512], dt16, kind="ExternalInput").ap()
    w2l_d = nc.dram_tensor("w2l", [128, 4 * 512], dt16, kind="ExternalInput").ap()
    w3l_d = nc.dram_tensor("w3l", [128, 4 * 512], dt16, kind="ExternalInput").ap()
    pj_d = nc.dram_tensor("pj", [128, 2 * 512], dt16, kind="ExternalInput").ap()
    b1_d = nc.dram_tensor("b1", [128, 4], dt32, kind="ExternalInput").ap()
    b2_d = nc.dram_tensor("b2", [128, 4], dt32, kind="ExternalInput").ap()
    b3_d = nc.dram_tensor("b3", [1, 512], dt16, kind="ExternalInput").ap()
    out_d = nc.dram_tensor("out", [Y0, HSH], dt32, kind="ExternalOutput").ap()

    # slot index for the s-th contraction block (skip halo slot 4)
    def xs(s):
        return s + 1 if s >= 4 else s

    with tile.TileContext(nc) as tc:
        with (
            tc.tile_pool(name="cst", bufs=1) as cst,
            tc.tile_pool(name="pn", bufs=3) as pnp,
            tc.tile_pool(name="wst", bufs=4) as wst,
            tc.tile_pool(name="wsx", bufs=4) as wsx,
            tc.tile_pool(name="psA", bufs=1, space="PSUM") as psA,
            tc.tile_pool(name="psB", bufs=2, space="PSUM") as psB,
            tc.tile_pool(name="dram", bufs=1, space="DRAM") as drp,
        ):
            # ---------- persistent SBUF loads (ring-conscious order) ----------
            # sync ring: aux -> xt(first 6 slots) -> AR staging -> ...
            aux_t = cst.tile([128, BPC * 3], dt16, tag="aux")
            nc.sync.dma_start(aux_t, aux_d)
            xt_t = cst.tile([128, NSLOT * 256], dt16, tag="xt")
            nc.sync.dma_start(xt_t[:, 0:6 * 256], xt_d[:, 0:6 * 256])
            # scalar ring: tp -> biases -> w1 stream
            tp_t = cst.tile([128, 512], dt16, tag="tp")
            nc.scalar.dma_start(tp_t, tp_d)
            b1_t = cst.tile([128, 4], dt32, tag="b1")
            nc.scalar.dma_start(b1_t, b1_d)
            b2_t = cst.tile([128, 4], dt32, tag="b2")
            nc.scalar.dma_start(b2_t, b2_d)
            b3_t = cst.tile([1, 512], dt16, tag="b3")
            nc.scalar.dma_start(b3_t, b3_d)
            # gpsimd ring: pj, local fc2/fc3 weights, xt rest
            pj_t = cst.tile([128, 2 * 512], dt16, tag="pj")
            nc.gpsimd.dma_start(pj_t, pj_d)
            w2l_t = cst.tile([128, 4 * 512], dt16, tag="w2l")
            nc.gpsimd.dma_start(w2l_t, w2l_d)
            w3l_t = cst.tile([128, 4 * 512], dt16, tag="w3l")
            nc.gpsimd.dma_start(w3l_t, w3l_d)
            half = (NSLOT * 256 - 6 * 256) // 2 + 6 * 256
            nc.gpsimd.dma_start(xt_t[:, 6 * 256:half], xt_d[:, 6 * 256:half])
            nc.gpsimd.dma_start(xt_t[:, half:], xt_d[:, half:])

            ones_t = cst.tile([128, 1], dt16, tag="ones")
            nc.vector.memset(ones_t, 1.0)
            onesr_t = cst.tile([1, 128], dt16, tag="onesr")
            nc.vector.memset(onesr_t, 1.0)
            # prewarm the gelu activation table on the scalar engine
            warm_t = cst.tile([1, 1], dt32, tag="warm")
            nc.vector.memset(warm_t, 0.5)
            nc.scalar.activation(warm_t, warm_t, GELU)

            # bounce buffers (DRAM)
            arA_i = drp.tile([261, NN], dt16, tag="arA_i")
            arA_o = drp.tile([261, NN], dt16, tag="arA_o", addr_space="Shared")
            g1_i = drp.tile([HSH, NN], dt16, tag="g1_i")
            g1_o = drp.tile([HID, NN], dt16, tag="g1_o", addr_space="Shared")
            g2_i = drp.tile([HSH, NN], dt16, tag="g2_i")
            g2_o = drp.tile([HID, NN], dt16, tag="g2_o", addr_space="Shared")

            # ---------- stage 1: cov partials over local time blocks ----------
            # u_ps[ic]: UT chunk [128*ic.., :] ; vecps rows 0:3 = (r,a,c),
            # row 32 = p, row 64 = q  (three partition groups, one bank)
            u_ps = [psA.tile([128, 256], dt32, tag=f"a{ic}", name=f"u{ic}")
                    for ic in range(2)]
            vecps = psA.tile([65, 256], dt32, tag="a2", name="vecps")
            for bl in range(BPC):
                xb = xt_t[:, 256 * bl: 256 * bl + 256]
                xb1 = xt_t[:, 256 * (bl + 1): 256 * (bl + 1) + 256]
                first, last = bl == 0, bl == BPC - 1
                pt_ps = psB.tile([128, 256], dt32, tag="rot", name="pt_ps")
                mm(pt_ps, tp_t[:, 0:128], xb, start=True, stop=False)
                mm(pt_ps, tp_t[:, 128:256], xb1, start=False, stop=True)
                pt_sb = pnp.tile([128, 256], dt16, tag="ptsb", name="pt_sb")
                nc.vector.tensor_copy(pt_sb, pt_ps)
                nt_ps = psB.tile([128, 256], dt32, tag="rot", name="nt_ps")
                mm(nt_ps, tp_t[:, 256:384], xb, start=True, stop=False)
                mm(nt_ps, tp_t[:, 384:512], xb1, start=False, stop=True)
                nt_sb = pnp.tile([128, 256], dt16, tag="ntsb", name="nt_sb")
                nc.vector.tensor_copy(nt_sb, nt_ps)
                for ic in range(2):
                    xbc = xt_t[:, 256 * bl + 128 * ic: 256 * bl + 128 * ic + 128]
                    mm(u_ps[ic], pt_sb[:, 128 * ic:128 * ic + 128], xb,
                       start=first, stop=False)
                    mm(u_ps[ic], xbc, nt_sb, start=False, stop=last)
                mm(vecps[0:3, :], aux_t[:, 3 * bl:3 * bl + 3], xb,
                   start=first, stop=last)
                mm(vecps[32:33, :], ones_t, pt_sb, start=first, stop=last)
                mm(vecps[64:65, :], ones_t, nt_sb, start=first, stop=last)

            # pack to bounce + AllReduce (bf16): rows 0:256 = UT/D,
            # 256..260 = r,a,c,p,q (raw)
            stgs = []
            for ic in range(2):
                stg = cst.tile([128, 256], dt16, tag=f"stg{ic}", name=f"stg{ic}")
                nc.vector.tensor_scalar_mul(stg, u_ps[ic], 1.0 / DD)
                stgs.append(stg)
            vec3 = cst.tile([3, 256], dt16, tag="vec3", name="vec3")
            nc.vector.tensor_copy(vec3, vecps[0:3, :])
            p_sb = cst.tile([1, 256], dt16, tag="p_sb", name="p_sb")
            nc.vector.tensor_copy(p_sb, vecps[32:33, :])
            q_sb = cst.tile([1, 256], dt16, tag="q_sb", name="q_sb")
            nc.vector.tensor_copy(q_sb, vecps[64:65, :])
            nc.sync.dma_start(arA_i[0:128, :], stgs[0])
            nc.sync.dma_start(arA_i[128:256, :], stgs[1])
            nc.sync.dma_start(arA_i[256:259, :], vec3)
            nc.sync.dma_start(arA_i[259:260, :], p_sb)
            nc.sync.dma_start(arA_i[260:261, :], q_sb)
            nc.gpsimd.collective_compute(
                "AllReduce", mybir.AluOpType.add, replica_groups=RG,
                ins=[arA_i.opt()], outs=[arA_o.opt()])

            # ---------- stage 2b: fc1 X-half (overlaps AllReduce) ----------
            f1_tags = ["a3", "a4", "a5", "a0"]
            f1_ps = [psA.tile([128, 256], dt32, tag=f1_tags[hh], name=f"f1_{hh}")
                     for hh in range(4)]
            for c4 in range(8):
                wt = wsx.tile([128, 2048], dt16, tag="wx", name="wt")
                nc.scalar.dma_start(wt, w1_d[:, 2048 * c4: 2048 * c4 + 2048])
                for dk in range(4):
                    s = 4 * c4 + dk
                    xo = 256 * xs(s)
                    for hh in range(4):
                        mm(f1_ps[hh],
                           wt[:, 512 * dk + 128 * hh: 512 * dk + 128 * hh + 128],
                           xt_t[:, xo:xo + 256],
                           start=(s == 0), stop=False)

            # ---------- stage 2: G = X @ W1c (cov-independent, in AR shadow)
            g_tags = ["a1", "a2"]
            g_ps = [psA.tile([128, 512], dt32, tag=g_tags[ib], name=f"g_{ib}")
                    for ib in range(2)]
            for c4 in range(8):
                wt = wst.tile([128, 2048], dt16, tag="wc", name="wt")
                nc.scalar.dma_start(
                    wt, w1_d[:, 2048 * (8 + c4): 2048 * (8 + c4) + 2048])
                for dk in range(4):
                    s = 4 * c4 + dk
                    xo = 256 * xs(s)
                    for ib in range(2):
                        mm(g_ps[ib],
                           xt_t[:, xo + 128 * ib: xo + 128 * ib + 128],
                           wt[:, 512 * dk: 512 * dk + 512],
                           start=(s == 0), stop=(s == 31))
            gT = cst.tile([128, 2 * 512], dt16, tag="gT")
            for ib in range(2):
                nc.vector.tensor_copy(gT[:, 512 * ib:512 * ib + 512], g_ps[ib])

            # ---------- stage 3: corrections + covT ----------
            ured = cst.tile([128, 2 * 256], dt16, tag="ured", name="ured")
            nc.sync.dma_start(ured[:, 0:256], arA_o[0:128, :])
            nc.sync.dma_start(ured[:, 256:512], arA_o[128:256, :])
            vt5 = cst.tile([1, 5 * 256], dt16, tag="vt5", name="vt5")
            nc.sync.dma_start(vt5.rearrange("p (b c) -> p b c", b=5),
                              arA_o[256:261, :].unsqueeze(0))
            rr = vt5[:, 0:256]
            ra = vt5[:, 256:512]
            rc = vt5[:, 512:768]
            rp = vt5[:, 768:1024]
            rq = vt5[:, 1024:1280]
            # staged U is pre-divided by D -> alpha/beta carry the same 1/D.
            m16 = cst.tile([1, NN], dt16, tag="m16")
            nc.vector.tensor_scalar_mul(m16, rr, 1.0 / DD)
            al32 = cst.tile([1, NN], dt32, tag="al32")
            nc.vector.tensor_add(al32, ra, rq)
            nc.vector.tensor_scalar_mul(al32, al32, 1.0 / DD)
            gm32 = cst.tile([1, NN], dt32, tag="gm32")
            nc.vector.tensor_scalar_mul(gm32, rr, gamma / (DD * DD))
            al16 = cst.tile([1, NN], dt16, tag="al16")
            nc.vector.tensor_sub(al16, al32, gm32)
            be32 = cst.tile([1, NN], dt32, tag="be32")
            nc.vector.tensor_add(be32, rp, rc)
            be16 = cst.tile([1, NN], dt16, tag="be16")
            nc.vector.tensor_scalar_mul(be16, be32, 1.0 / DD)

            # gate: w2 stream starts only after the AllReduce has landed
            w2R = cst.tile([128, KB * 512], dt16, tag="w2R")
            nc.vector.tensor_copy(w2R[0:1, 0:1], m16[0:1, 0:1])
            for sp in range(4):
                w = KB * 512 // 4
                nc.scalar.dma_start(w2R[:, w * sp: w * (sp + 1)],
                                    w2_d[:, w * sp: w * (sp + 1)])

            covt = cst.tile([128, 2 * 256], dt16, tag="covt")
            for ic in range(2):
                corr = psB.tile([128, 256], dt32, tag="rot", name="corr")
                mm(corr, m16[:, 128 * ic:128 * ic + 128], al16,
                   start=True, stop=False)
                mm(corr, be16[:, 128 * ic:128 * ic + 128], m16,
                   start=False, stop=True)
                nc.vector.tensor_sub(covt[:, 256 * ic:256 * ic + 256],
                                     ured[:, 256 * ic:256 * ic + 256], corr)

            # ---------- stage 5: fc1 cov contribution = G @ covT + gelu ----
            for hh in range(4):
                for ib in range(2):
                    mm(f1_ps[hh], gT[:, 512 * ib + 128 * hh: 512 * ib + 128 * hh + 128],
                       covt[:, 256 * ib:256 * ib + 256],
                       start=False, stop=(ib == 1))
            a1loc = cst.tile([128, 4 * 256], dt16, tag="a1loc")
            for hh in range(4):
                nc.scalar.activation(a1loc[:, 256 * hh:256 * hh + 256],
                                     f1_ps[hh], GELU, bias=b1_t[:, hh:hh + 1])
                nc.sync.dma_start(g1_i[128 * hh:128 * hh + 128, :],
                                  a1loc[:, 256 * hh:256 * hh + 256])

            # ---------- stage 6: AllGather a1, batched unpack ----------
            nc.gpsimd.collective_compute(
                "AllGather", mybir.AluOpType.bypass, replica_groups=RG,
                ins=[g1_i.opt()], outs=[g1_o.opt()])
            a1f = cst.tile([128, NBLK * 256], dt16, tag="a1f")
            g1v = g1_o.rearrange("(b p) c -> p b c", p=128)
            a1v = a1f.rearrange("p (b c) -> p b c", b=NBLK)
            nc.sync.dma_start(a1v[:, 0:16, :], g1v[:, 0:16, :])
            nc.scalar.dma_start(a1v[:, 16:32, :], g1v[:, 16:32, :])

            # gate: w3 stream starts only after AG1 output started landing
            w3R = cst.tile([128, KB * 512], dt16, tag="w3R", name="w3R")
            nc.vector.tensor_copy(w3R[0:1, 0:1], a1f[0:1, 0:1])
            for sp in range(4):
                w = KB * 512 // 4
                nc.gpsimd.dma_start(w3R[:, w * sp: w * (sp + 1)],
                                    w3_d[:, w * sp: w * (sp + 1)])

            # ---------- stage 7: fc2 + gelu ----------
            f2_ps = [psA.tile([128, 256], dt32, tag=f1_tags[hh], name=f"f2_{hh}")
                     for hh in range(4)]
            for j in range(BPC):
                for hh in range(4):
                    mm(f2_ps[hh],
                       w2l_t[:, 512 * j + 128 * hh: 512 * j + 128 * hh + 128],
                       a1loc[:, 256 * j:256 * j + 256],
                       start=(j == 0), stop=False)
            for k in range(KB):
                for hh in range(4):
                    mm(f2_ps[hh],
                       w2R[:, 512 * k + 128 * hh: 512 * k + 128 * hh + 128],
                       a1f[:, 256 * k:256 * k + 256],
                       start=False, stop=(k == KB - 1))
            a2loc = cst.tile([128, 4 * 256], dt16, tag="a2loc")
            for hh in range(4):
                nc.scalar.activation(a2loc[:, 256 * hh:256 * hh + 256],
                                     f2_ps[hh], GELU, bias=b2_t[:, hh:hh + 1])
                nc.sync.dma_start(g2_i[128 * hh:128 * hh + 128, :],
                                  a2loc[:, 256 * hh:256 * hh + 256])

            # ---------- stage 8: AllGather a2, batched unpack ----------
            nc.gpsimd.collective_compute(
                "AllGather", mybir.AluOpType.bypass, replica_groups=RG,
                ins=[g2_i.opt()], outs=[g2_o.opt()])
            a2f = cst.tile([128, NBLK * 256], dt16, tag="a2f")
            g2v = g2_o.rearrange("(b p) c -> p b c", p=128)
            a2v = a2f.rearrange("p (b c) -> p b c", b=NBLK)
            nc.sync.dma_start(a2v[:, 0:16, :], g2v[:, 0:16, :])
            nc.scalar.dma_start(a2v[:, 16:32, :], g2v[:, 16:32, :])

            # ---------- stage 9: fc3 (batch-major out) ----------
            f3_ps = [psA.tile([128, 512], dt32, tag=g_tags[ii], name=f"f3_{ii}")
                     for ii in range(2)]
            o3_t = cst.tile([128, 2 * 512], dt16, tag="o3")
            for ii in range(2):
                for j in range(BPC):   # local chunk, runs during AllGather-2
                    mm(f3_ps[ii],
                       a2loc[:, 256 * j + 128 * ii: 256 * j + 128 * ii + 128],
                       w3l_t[:, 512 * j: 512 * j + 512],
                       start=(j == 0), stop=False)
                for k in range(KB):
                    mm(f3_ps[ii],
                       a2f[:, 256 * k + 128 * ii: 256 * k + 128 * ii + 128],
                       w3R[:, 512 * k: 512 * k + 512],
                       start=False, stop=False)
                mm(f3_ps[ii], onesr_t, b3_t, start=False, stop=True)
                nc.vector.tensor_copy(o3_t[:, 512 * ii:512 * ii + 512], f3_ps[ii])

            # ---------- stage 10: proj ----------
            for pp in range(4):
                po = psB.tile([128, 512], dt32, tag="rot", name="po")
                for ii in range(2):
                    mm(po, pj_t[:, 512 * ii + 128 * pp: 512 * ii + 128 * pp + 128],
                       o3_t[:, 512 * ii:512 * ii + 512],
                       start=(ii == 0), stop=(ii == 1))
                osb = cst.tile([128, 512], dt32, tag=f"osb{pp}", name=f"osb{pp}")
                nc.vector.tensor_copy(osb, po)
                eng = nc.sync if pp % 2 == 0 else nc.scalar
                eng.dma_start(out_d[128 * pp:128 * pp + 128, :], osb)

    nc.compile()
    return nc


# ----------------------------------------------------------------------------
# host-side sharding / packing
# ----------------------------------------------------------------------------
def prep_in_maps(inputs):
    X = np.asarray(inputs["X"], np.float32)
    A0, A1, B0, B1, g_p, g_n, gamma = _filters(inputs)

    XT = np.ascontiguousarray(X.T)                      # [D, N]
    XTb = XT.reshape(NBLK, 128, NN)                     # [32, 128, 256]
    tp = np.concatenate([A0, A1, B0, B1], axis=1).astype(bf16)
    pjT = np.asarray(inputs["proj"], np.float64).T      # [256, 512]
    pj = pjT.reshape(2, 128, 512).transpose(1, 0, 2).reshape(128, 1024).astype(bf16)

    f64 = lambda k: np.asarray(inputs[k], np.float64)
    fc_wT = {1: f64("fc1_w").T, 2: f64("fc2_w").T, 3: f64("fc3_w").T}

    in_maps = []
    for c in range(NCORES):
        # xt: 33 slots: 0-3 local blocks, 4 halo (zeros for last core),
        # 5-32 the remaining blocks in rotated global order
        order = [(4 * c + i) % NBLK for i in range(4)]
        halo = (4 * c + 4) % NBLK
        rest = [(4 * c + 4 + i) % NBLK for i in range(NBLK - 4)]
        xtb = np.zeros((128, NSLOT, NN), np.float32)
        for sl, gb in enumerate(order):
            xtb[:, sl, :] = XTb[gb]
        if c != NCORES - 1:
            xtb[:, 4, :] = XTb[halo]
        for i, gb in enumerate(rest):
            xtb[:, 5 + i, :] = XTb[gb]
        xt = xtb.reshape(128, NSLOT * NN).astype(bf16)

        aux = np.zeros((128, BPC * 3), np.float32)
        for bl in range(BPC):
            gb = 4 * c + bl
            aux[:, 3 * bl + 0] = 1.0
            aux[:, 3 * bl + 1] = g_p[128 * gb:128 * gb + 128]
            aux[:, 3 * bl + 2] = g_n[128 * gb:128 * gb + 128]

        hs = slice(HSH * c, HSH * (c + 1))
        korder = order + rest                         # contraction block order
        # w1: X half then cov half, k-blocks in the same rotated order as xt
        w1x = fc_wT[1][0:DD, hs].reshape(NBLK, 128, HSH)[korder]
        w1c = fc_wT[1][DD:2 * DD, hs].reshape(NBLK, 128, HSH)[korder]
        w1 = np.concatenate([w1x, w1c], axis=0).transpose(1, 0, 2) \
            .reshape(128, 64 * HSH).astype(bf16)
        w2full = fc_wT[2][:, hs].reshape(KB, 128, HSH)
        w3full = fc_wT[3][:, hs].reshape(KB, 128, HSH)
        lb = slice(BPC * c, BPC * (c + 1))       # this core's local k-blocks
        w2l = w2full[lb].transpose(1, 0, 2).reshape(128, BPC * HSH).astype(bf16)
        w3l = w3full[lb].transpose(1, 0, 2).reshape(128, BPC * HSH).astype(bf16)
        w2full = w2full.copy(); w2full[lb] = 0.0
        w3full = w3full.copy(); w3full[lb] = 0.0
        w2 = w2full.transpose(1, 0, 2).reshape(128, KB * HSH).astype(bf16)
        w3 = w3full.transpose(1, 0, 2).reshape(128, KB * HSH).astype(bf16)
        b1 = f64("fc1_b")[hs].reshape(4, 128).T.astype(np.float32)
        b2 = f64("fc2_b")[hs].reshape(4, 128).T.astype(np.float32)
        b3 = f64("fc3_b")[hs].reshape(1, HSH).astype(bf16)
        in_maps.append({
            "xt": xt, "tp": tp, "aux": aux.astype(bf16),
            "w1": w1, "w2": w2, "w3": w3, "w2l": w2l, "w3l": w3l,
            "pj": pj, "b1": b1, "b2": b2, "b3": b3,
        })
    return in_maps, gamma


def run(inputs, trace=False, **kw):
    in_maps, gamma = prep_in_maps(inputs)
    key = ("nc", float(gamma))
    if key not in _CACHE:
        _CACHE[key] = build(gamma)
    nc = _CACHE[key]
    from concourse import bass_utils
    res = bass_utils.run_bass_kernel_spmd(nc, in_maps,
                                          core_ids=list(range(NCORES)),
                                          trace=trace, **kw)
    out = np.concatenate([res.results[c]["out"] for c in range(NCORES)], axis=1)
    return out.astype(np.float32), res


def kernel(**inputs) -> np.ndarray:
    out, _ = run(inputs)
    return out


if __name__ == "__main__":
    data = np.load("inputs.npz")
    inputs = {k: data[k] for k in data.files}
    expected = np.load("expected.npy")
    out = kernel(**inputs)
    scale = np.abs(expected).max()
    err = np.abs(out - expected).max() / scale
    print(f"Relative error: {err:.3e}")
